# revision 2
# baseline (speedup 1.0000x reference)
"""Trainium2 Bass kernel v3: node-sharded linear attention with AllReduce.

v2 (baseline) computed Phase A (k/v proj + ktv + sumk over ALL N=32768
nodes) redundantly on every core to avoid collectives. That made Phase A
~68.7G of the ~79.4G MACs per core.

v3 shards Phase A over nodes too: each core computes phi_k^T v and
sum(phi_k) partials over its OWN 4096 source nodes, then a single 2.07MB
AllReduce (ktv [H,2,2,128,128-lhsT-chunks] + sumk columns [128,16]) sums
partials across the 8 cores. Phase B (q proj, phi_q, numerator /
denominator, v_map path, final projection, Lorentz lift) is node-local
as before. Per-core MACs drop ~4.3x to ~18.3G.

Timing reps are python-unrolled (collectives cannot re-execute inside
rolled hardware loops on this stack); the graded kernel() entry point
builds the single-rep program.

All matmuls run as float32r (full PE rate at moving-dim>=256).
"""

import os
import numpy as np
import ml_dtypes
import concourse.bass as bass
import concourse.tile as tile
from concourse import bacc, mybir
from concourse.bass_utils import run_bass_kernel_spmd

F32 = mybir.dt.float32
F32R = mybir.dt.float32r
BF16 = mybir.dt.bfloat16
AF = mybir.ActivationFunctionType
ALU = mybir.AluOpType

NCORES = 8
N = 32768
NCHUNK = N // NCORES          # 4096 nodes per core
H = 8
D = 256
HD = H * D                    # 2048
KC = 3                        # contraction chunks: 384 = 3*128 (258 used)
EPS = 1e-6
CCG = 2048 + 64               # per-group AllGather width: 4-head ktv + sumk

_CACHE = {}


def _build(reps=1):
    key = (reps, os.environ.get("KT_DEBUG"), os.environ.get("KT_UNROLL"),
           os.environ.get("KT_NOSIDES"), os.environ.get("KT_WPA_BUFS"),
           os.environ.get("KT_SKIP_A"), os.environ.get("KT_SKIP_B"),
           os.environ.get("KT_SKIP_CC"), os.environ.get("KT_NTILES"),
           os.environ.get("KT_NST"))
    if key in _CACHE:
        return _CACHE[key]
    nc = bacc.Bacc("TRN2", target_bir_lowering=False, debug=False,
                   num_devices=NCORES)

    xqT = nc.dram_tensor("xqT", [KC, 128, NCHUNK], F32R, kind="ExternalInput").ap()
    xbT = nc.dram_tensor("xbT", [KC, 128, NCHUNK], F32R, kind="ExternalInput").ap()
    wq = nc.dram_tensor("wq", [KC, 128, HD], F32R, kind="ExternalInput").ap()
    wk = nc.dram_tensor("wk", [KC, 128, HD], F32R, kind="ExternalInput").ap()
    wv = nc.dram_tensor("wv", [KC, 128, HD], F32R, kind="ExternalInput").ap()
    wvm = nc.dram_tensor("wvm", [KC, 128, HD], F32R, kind="ExternalInput").ap()
    fw = nc.dram_tensor("fw", [16, 128, D], BF16, kind="ExternalInput").ap()
    fbias = nc.dram_tensor("fbias", [1, D], F32R, kind="ExternalInput").ap()
    ones_r = nc.dram_tensor("ones_r", [1, 128], F32R, kind="ExternalInput").ap()
    ones_c = nc.dram_tensor("ones_c", [128, 8], F32R, kind="ExternalInput").ap()
    ind = nc.dram_tensor("ind", [128, 8, 8], BF16, kind="ExternalInput").ap()
    indr = nc.dram_tensor("indr", [1, 8, 8], F32R, kind="ExternalInput").ap()
    ind2 = nc.dram_tensor("ind2", [8, 8, 128], F32R, kind="ExternalInput").ap()
    cons = nc.dram_tensor("cons", [8, 1], F32, kind="ExternalInput").ap()
    out = nc.dram_tensor("out", [NCHUNK, 257], F32, kind="ExternalOutput").ap()
    dbg = (nc.dram_tensor("dbg", [128, CCW], F32, kind="ExternalOutput").ap()
           if os.environ.get("KT_DEBUG") else None)

    with tile.TileContext(nc) as tc:
        _body(nc, tc, reps, xqT, xbT, wq, wk, wv, wvm, fw, fbias,
              ones_r, ones_c, ind, indr, ind2, cons, out, dbg)
    nc.compile()
    _CACHE[key] = nc
    return nc


def _body(nc, tc, reps, xqT, xbT, wq, wk, wv, wvm, fw, fbias,
          ones_r, ones_c, ind, indr, ind2, cons, out, dbg=None):
    import contextlib
    stack = contextlib.ExitStack()
    with stack:
        cpool = stack.enter_context(tc.tile_pool(name="const", bufs=1))
        dpool = stack.enter_context(tc.tile_pool(name="dramcc", bufs=1,
                                                 space="DRAM"))

        ones_r_sb = cpool.tile([1, 128], F32R)
        nc.sync.dma_start(ones_r_sb[:], ones_r[:])
        ones_c_sb = cpool.tile([128, 8], F32R)
        nc.sync.dma_start(ones_c_sb[:], ones_c[:])
        ind_sb = cpool.tile([128, 8, 8], BF16)
        nc.sync.dma_start(ind_sb[:], ind[:])
        indr_sb = cpool.tile([1, 8, 8], F32R)
        nc.sync.dma_start(indr_sb[:], indr[:])
        ind2_sb = cpool.tile([8, 8, 128], F32R)
        nc.sync.dma_start(ind2_sb[:], ind2[:])
        fb_sb = cpool.tile([1, D], F32R)
        nc.sync.dma_start(fb_sb[:], fbias[:])
        eps_sb = cpool.tile([8, 1], F32)
        nc.sync.dma_start(eps_sb[:], cons[:])

        # phase A -> AllGather -> phase B carriers
        # ktv as lhsT chunks [m_loc, h, mc, dc, d_loc]. Phase A emits f32
        # partials per head-group; each group's partials AllGather while the
        # next group computes; local f32 sum + bf16 rounding follow.
        # bufs=2 so rep r+1's phase A does not WAR-stall on rep r's phase B
        # still reading the previous carriers.
        kpool = stack.enter_context(tc.tile_pool(name="kcar", bufs=2))
        gpool = stack.enter_context(tc.tile_pool(name="agrp", bufs=2))
        agsp = stack.enter_context(tc.tile_pool(name="agstg", bufs=2))
        agap = stack.enter_context(tc.tile_pool(name="agacc", bufs=2))

        # phase B weights are rep-invariant: load once, keep resident
        wq_sb = cpool.tile([128, KC, HD], F32R)
        nc.sync.dma_start(wq_sb[:], wq.rearrange("c p n -> p c n"))
        wvm_sb = cpool.tile([128, KC, HD], F32R)
        nc.sync.dma_start(wvm_sb[:], wvm.rearrange("c p n -> p c n"))
        fw_sb = cpool.tile([128, 16, D], BF16)
        nc.sync.dma_start(fw_sb[:], fw.rearrange("c p n -> p c n"))

        def group_done(r, g, ktv_g, sumk_g, ktv_sb, sumk_w):
            """Called by _phase_a when head-group g's partials are ready.
            AllGather (pure copy), NOT AllReduce: the CCE reduction datapath
            rounds (~6.6e-4 max rel err measured); summing locally in f32
            keeps the 8-partial reduction exact. Collectives cannot
            re-execute inside rolled loops and a Shared output allows a
            single writer inst, so each (rep, group) gets its own bounce."""
            if os.environ.get("KT_SKIP_CC"):
                nc.scalar.copy(
                    ktv_sb[:, g * 4:(g + 1) * 4].rearrange(
                        "p h mc dc dl -> p (h mc dc dl)"), ktv_g[:])
                nc.scalar.copy(
                    sumk_w[:, g * 8:(g + 1) * 8].rearrange("p c h -> p (c h)"),
                    sumk_g[:])
                return
            cc_in = dpool.tile([128, CCG], F32, tag=f"ccin{r}_{g}",
                               name=f"ccin{r}_{g}")
            cc_out = dpool.tile([NCORES * 128, CCG], F32, addr_space="Shared",
                                tag=f"ccout{r}_{g}", name=f"ccout{r}_{g}")
            nc.sync.dma_start(cc_in[:, 0:2048], ktv_g[:])
            nc.sync.dma_start(cc_in[:, 2048:CCG], sumk_g[:])
            nc.gpsimd.collective_compute(
                "AllGather", ALU.bypass,
                replica_groups=[list(range(NCORES))],
                ins=[cc_in[:]], outs=[cc_out[:]])
            # local exact-f32 sum of the 8 gathered partials. One folded
            # DMA per chunk pulls all 8 ranks' [128, QW] slices (ranks
            # stack on the partition axis of cc_out), then 7 DVE adds.
            ktv_flat = ktv_sb[:, g * 4:(g + 1) * 4].rearrange(
                "p h mc dc dl -> p (h mc dc dl)")
            sumk_flat = sumk_w[:, g * 8:(g + 1) * 8].rearrange(
                "p c h -> p (c h)")
            cc_rk = cc_out.rearrange("(r p) w -> p r w", r=NCORES)
            QW = CCG // 16
            for q in range(16):
                lo, hi = q * QW, (q + 1) * QW
                stg = agsp.tile([128, NCORES, QW], F32, tag="agstg")
                nc.sync.dma_start(stg[:], cc_rk[:, :, lo:hi])
                acc = agap.tile([128, QW], F32, tag="agacc")
                nc.vector.tensor_add(acc[:], stg[:, 0], stg[:, 1])
                for rk in range(2, NCORES):
                    nc.vector.tensor_add(acc[:], acc[:], stg[:, rk])
                if hi <= 2048:
                    nc.scalar.copy(ktv_flat[:, lo:hi], acc[:])
                else:
                    nc.scalar.copy(ktv_flat[:, lo:2048], acc[:, 0:2048 - lo])
                    nc.scalar.copy(sumk_flat[:], acc[:, 2048 - lo:])

        def rep_body(r):
            ktv_sb = kpool.tile([128, H, 2, 2, 128], BF16, tag="ktvsb",
                                name="ktv_sb")
            sumk_w = kpool.tile([128, 16, 8], BF16, tag="sumkw",
                                name="sumk_w")
            if not os.environ.get("KT_SKIP_A"):
                _phase_a(nc, tc, xbT, wk, wv, ones_c_sb, indr_sb, gpool,
                         lambda g, kg, sg: group_done(r, g, kg, sg,
                                                      ktv_sb, sumk_w))
            else:
                # timing-only ablation: garbage-init the carriers
                wsrc = wk.rearrange("c p n -> p c n")
                for g in range(2):
                    ktv_g = gpool.tile([128, 2048], F32, tag="ktvg")
                    sumk_g = gpool.tile([128, 64], F32, tag="sumkg")
                    nc.gpsimd.dma_start(ktv_g[:], wsrc[:, g])
                    nc.gpsimd.dma_start(sumk_g[:], wsrc[:, 2, 0:64])
                    group_done(r, g, ktv_g, sumk_g, ktv_sb, sumk_w)
            if not os.environ.get("KT_SKIP_B"):
                _phase_b(nc, tc, xqT, xbT, wq_sb, wvm_sb, fw_sb, fb_sb,
                         ones_r_sb, ind_sb, ind2_sb, eps_sb,
                         ktv_sb, sumk_w, out)

        # For_i is only legal when the collective is skipped or reps==1;
        # timing builds use python-unrolled reps (KT_UNROLL)
        if os.environ.get("KT_UNROLL") or reps == 1:
            for r in range(reps):
                rep_body(r)
        else:
            assert os.environ.get("KT_SKIP_CC"), (
                "rolled reps>1 require KT_SKIP_CC (collectives can't loop)")
            with tc.For_i(0, reps, name="reploop"):
                rep_body(0)


def _phase_a(nc, tc, xsT, wk, wv, ones_c_sb, indr_sb, gpool, group_done):
    """Per head-group (4 heads): project k/v for own nodes, accumulate ktv in
    PSUM and sumk in SBUF, reduce/transpose, emit f32 partials via
    group_done(g, ktv_g[128,2048], sumk_g[128,64]) so the group's AllGather
    overlaps the next group's compute."""
    import contextlib
    sd = None if os.environ.get("KT_NOSIDES") else "left"
    wpb = int(os.environ.get("KT_WPA_BUFS", 2))
    with contextlib.ExitStack() as st:
        apool = st.enter_context(tc.tile_pool(name="accA", bufs=2, side=sd))
        wpA = st.enter_context(tc.tile_pool(name="wA", bufs=wpb, side=sd))
        xp = st.enter_context(tc.tile_pool(name="xA", bufs=3, side=sd))
        zp = st.enter_context(tc.tile_pool(name="zA", bufs=2, side=sd))
        yp = st.enter_context(tc.tile_pool(name="yA", bufs=2, side=sd))
        scrp = st.enter_context(tc.tile_pool(name="scrA", bufs=2, side=sd))
        stp = st.enter_context(tc.tile_pool(name="stA", bufs=4, side=sd))
        php = st.enter_context(tc.tile_pool(name="phA", bufs=3, side=sd))
        vp = st.enter_context(tc.tile_pool(name="vA", bufs=3, side=sd))
        pk = st.enter_context(tc.tile_pool(name="psAk", bufs=1, space="PSUM"))
        pp = st.enter_context(tc.tile_pool(name="psAp", bufs=3, space="PSUM"))
        psk = st.enter_context(tc.tile_pool(name="psAs", bufs=1, space="PSUM"))

        def ktv_mms(ktv_ps, phi, v_sb, first, last):
            # ktv[h][m,d] += phi[:, h*256+mc*128]^T v[:, h*256:+256]
            # PSUM: start=True clears has_written for the WHOLE bank (= one
            # hh's 512 cols here), so only the mc=0 group may clear; mc=1's
            # first write lands on cleared bits and overwrites correctly.
            for hh in range(4):
                for mc in range(2):
                    nc.tensor.matmul(
                        ktv_ps[:, hh, mc * 256: mc * 256 + 256],
                        lhsT=phi[:, hh * 256 + mc * 128: hh * 256 + mc * 128 + 128],
                        rhs=v_sb[:, hh * 256: hh * 256 + 256],
                        start=(first and mc == 0), stop=last)

        ntiles = int(os.environ.get("KT_NTILES", NCHUNK // 128))
        wk_r = wk.rearrange("c p n -> p c n")
        wv_r = wv.rearrange("c p n -> p c n")
        for g in range(2):
            gofs = g * 1024
            wk_g = wpA.tile([128, KC, 1024], F32R, tag="wkg")
            nc.sync.dma_start(wk_g[:], wk_r[:, :, gofs:gofs + 1024])
            wv_g = wpA.tile([128, KC, 1024], F32R, tag="wvg")
            nc.sync.dma_start(wv_g[:], wv_r[:, :, gofs:gofs + 1024])
            sumk_acc = apool.tile([128, 1024], F32R, tag="ska")

            ktv_ps = pk.tile([128, 4, 512], F32)
            prev = None
            for t in range(ntiles):
                xs_sb = xp.tile([128, KC, 128], F32R, tag="xs")
                nc.sync.dma_start(
                    xs_sb[:],
                    xsT[:, :, t * 128:(t + 1) * 128].rearrange("c p n -> p c n"))

                ks_ps = []
                vs_ps = []
                for blk in range(2):
                    kp_t = pp.tile([128, 512], F32, tag="projA")
                    for c in range(KC):
                        nc.tensor.matmul(
                            kp_t[:], lhsT=xs_sb[:, c],
                            rhs=wk_g[:, c, blk * 512: blk * 512 + 512],
                            start=(c == 0), stop=(c == KC - 1))
                    ks_ps.append(kp_t)
                for blk in range(2):
                    vp_t = pp.tile([128, 512], F32, tag="projA")
                    for c in range(KC):
                        nc.tensor.matmul(
                            vp_t[:], lhsT=xs_sb[:, c],
                            rhs=wv_g[:, c, blk * 512: blk * 512 + 512],
                            start=(c == 0), stop=(c == KC - 1))
                    vs_ps.append(vp_t)

                # lagged ktv for the previous tile: keeps the PE busy while
                # this tile's z/y/phi chain runs on DVE/ACT
                if prev is not None:
                    ktv_mms(ktv_ps, prev[0], prev[1], prev[2] == 0, False)

                # z = relu(ks) + eps
                z = zp.tile([128, 1024], F32, tag="z")
                for blk in range(2):
                    nc.vector.tensor_scalar(
                        z[:, blk * 512:(blk + 1) * 512], ks_ps[blk][:],
                        0.0, EPS, ALU.max, ALU.add)
                # v copy to SBUF on DVE (ACT is the bottleneck engine)
                v_sb = vp.tile([128, 1024], BF16, tag="v")
                for blk in range(2):
                    nc.vector.tensor_copy(v_sb[:, blk * 512:(blk + 1) * 512],
                                          vs_ps[blk][:])

                # y = z^2 with per-head accumulated sums
                y = yp.tile([128, 1024], BF16, tag="y")
                sy = stp.tile([128, 4], F32, tag="sy")
                sy2 = stp.tile([128, 4], F32, tag="sy2")
                for hh in range(4):
                    sl = slice(hh * 256, hh * 256 + 256)
                    nc.scalar.activation(y[:, sl], z[:, sl], AF.Square,
                                         accum_out=sy[:, hh:hh + 1])
                for hh in range(4):
                    sl = slice(hh * 256, hh * 256 + 256)
                    scr = scrp.tile([128, 256], BF16, tag="y2scr")
                    nc.vector.tensor_mul(scr[:], y[:, sl], y[:, sl])
                    nc.vector.tensor_reduce(sy2[:, hh:hh + 1], scr[:],
                                            mybir.AxisListType.X, ALU.add)
                # factor = sqrt(sy / sy2)
                rec = stp.tile([128, 4], F32, tag="rec")
                nc.vector.reciprocal(rec[:], sy2[:])
                rat = stp.tile([128, 4], F32, tag="rat")
                nc.vector.tensor_mul(rat[:], sy[:], rec[:])
                fac = stp.tile([128, 4], F32, tag="fac")
                nc.scalar.activation(fac[:], rat[:], AF.Sqrt)

                phi = php.tile([128, 1024], BF16, tag="phi")
                for hh in range(4):
                    sl = slice(hh * 256, hh * 256 + 256)
                    nc.vector.tensor_scalar_mul(phi[:, sl], y[:, sl],
                                                fac[:, hh:hh + 1])
                # sumk accumulation
                if t == 0:
                    nc.scalar.copy(sumk_acc[:], phi[:])
                else:
                    nc.vector.tensor_add(sumk_acc[:], sumk_acc[:].bitcast(F32),
                                         phi[:])

                prev = (phi, v_sb, t)

            # tail: ktv for the final tile closes the accumulation group
            ktv_mms(ktv_ps, prev[0], prev[1], prev[2] == 0, True)

            # drain ktv psum into the f32 partial for this group's AllGather
            ktv_g = gpool.tile([128, 2048], F32, tag="ktvg")
            for hh in range(4):
                nc.scalar.copy(ktv_g[:, hh * 512:(hh + 1) * 512], ktv_ps[:, hh])

            # sumk partition-reduction for this group: [128, 1024] -> [1, 1024]
            srow = apool.tile([1, 1024], F32R, tag="srow")
            for blk in range(2):
                scr = psk.tile([128, 512], F32, tag="pscr")
                nc.tensor.matmul(
                    scr[0:8, :], lhsT=ones_c_sb[:],
                    rhs=sumk_acc[:, blk * 512:(blk + 1) * 512],
                    start=True, stop=True)
                nc.scalar.copy(srow[:, blk * 512:(blk + 1) * 512], scr[0:1, :])

            # transpose each 128-chunk of srow into sumk_g[:, j, :]
            # (col h(c) = sumk, rest zero), c = g*8 + j
            sumk_g = gpool.tile([128, 8, 8], F32, tag="sumkg")
            for j in range(8):
                hh = g * 4 + j // 2
                scr = psk.tile([128, 512], F32, tag="pscr")
                nc.tensor.matmul(scr[:, 0:8], lhsT=srow[:, j * 128:(j + 1) * 128],
                                 rhs=indr_sb[0:1, hh, :], start=True, stop=True)
                nc.scalar.copy(sumk_g[:, j], scr[:, 0:8])

            group_done(g, ktv_g,
                       sumk_g.rearrange("p c h -> p (c h)"))


def _phase_b(nc, tc, xqT, xbT, wq_sb, wvm_sb, fw_sb, fb_sb, ones_r_sb,
             ind_sb, ind2_sb, eps_sb, ktv_sb, sumk_w, out):
    import contextlib
    with contextlib.ExitStack() as st:
        sd = None if os.environ.get("KT_NOSIDES") else "right"
        xp = st.enter_context(tc.tile_pool(name="xB", bufs=2, side=sd))
        zp = st.enter_context(tc.tile_pool(name="zB", bufs=3, side=sd))
        yp = st.enter_context(tc.tile_pool(name="yB", bufs=17, side=sd))
        y2p = st.enter_context(tc.tile_pool(name="y2B", bufs=3, side=sd))
        stp = st.enter_context(tc.tile_pool(name="stB", bufs=2, side=sd))
        php = st.enter_context(tc.tile_pool(name="phB", bufs=17, side=sd))
        atp = st.enter_context(tc.tile_pool(name="atB", bufs=17, side=sd))
        obp = st.enter_context(tc.tile_pool(name="oB", bufs=3, side=sd))
        qp = st.enter_context(tc.tile_pool(name="psBq", bufs=2, space="PSUM"))
        sump = st.enter_context(tc.tile_pool(name="psBs", bufs=1, space="PSUM"))
        sbp = st.enter_context(tc.tile_pool(name="psBb", bufs=1, space="PSUM"))
        ap_ = st.enter_context(tc.tile_pool(name="psBa", bufs=2, space="PSUM"))
        op = st.enter_context(tc.tile_pool(name="psBo", bufs=1, space="PSUM"))

        NST = 256                      # supertile node count
        nst = int(os.environ.get("KT_NST", NCHUNK // NST))
        for stx in range(nst):
            nofs = stx * NST
            xq_sb = xp.tile([128, KC, NST], F32R, tag="xq")
            nc.sync.dma_start(
                xq_sb[:], xqT[:, :, nofs:nofs + NST].rearrange("c p n -> p c n"))
            xs_sb = xp.tile([128, KC, NST], F32R, tag="xsB")
            nc.sync.dma_start(
                xs_sb[:], xbT[:, :, nofs:nofs + NST].rearrange("c p n -> p c n"))

            sums_ps = sump.tile([8, 3, NST], F32, tag="sums")

            def sums_mms(c, y_c, y2):
                # streams 0 and 1 share a PSUM bank: only stream 0 clears
                # (start=True wipes the whole bank's has_written bits);
                # stream 1's first write overwrites-on-clear correctly
                nc.tensor.matmul(sums_ps[:, 0], lhsT=ind_sb[:, c // 2], rhs=y_c[:],
                                 start=(c == 0), stop=(c == 15))
                nc.tensor.matmul(sums_ps[:, 1], lhsT=ind_sb[:, c // 2], rhs=y2[:],
                                 start=False, stop=(c == 15))
                nc.tensor.matmul(sums_ps[:, 2], lhsT=sumk_w[:, c], rhs=y_c[:],
                                 start=(c == 0), stop=(c == 15))

            ys = []
            pend = None
            for c in range(16):
                q_ps = qp.tile([128, NST], F32, tag="qps")
                for kc in range(KC):
                    nc.tensor.matmul(
                        q_ps[:], lhsT=wq_sb[:, kc, c * 128:(c + 1) * 128],
                        rhs=xq_sb[:, kc], start=(kc == 0), stop=(kc == KC - 1))
                # stats matmuls lag one chunk so the PE doesn't wait on the
                # z -> y -> y^2 chain of the chunk it just projected
                if pend is not None:
                    sums_mms(*pend)
                z = zp.tile([128, NST], F32, tag="zB")
                nc.vector.tensor_scalar(z[:], q_ps[:], 0.0, EPS, ALU.max, ALU.add)
                y_c = yp.tile([128, NST], BF16, tag="yB")
                nc.scalar.activation(y_c[:], z[:], AF.Square)
                y2 = y2p.tile([128, NST], BF16, tag="y2B")
                nc.vector.tensor_mul(y2[:], y_c[:], y_c[:])
                pend = (c, y_c, y2)
                ys.append(y_c)
            sums_mms(*pend)

            # stats on [8, NST]
            rec2 = stp.tile([8, NST], F32, tag="rec2")
            nc.vector.reciprocal(rec2[:], sums_ps[:, 1])
            rat = stp.tile([8, NST], F32, tag="ratB")
            nc.vector.tensor_mul(rat[:], sums_ps[:, 0], rec2[:])
            fac = stp.tile([8, NST], F32, tag="facB")
            nc.scalar.activation(fac[:], rat[:], AF.Sqrt)
            den = stp.tile([8, NST], F32, tag="den")
            nc.vector.tensor_mul(den[:], sums_ps[:, 2], fac[:])
            nc.vector.tensor_scalar_add(den[:], den[:], eps_sb[:])
            rden = stp.tile([8, NST], F32, tag="rden")
            nc.vector.reciprocal(rden[:], den[:])
            s_sb = stp.tile([8, NST], F32R, tag="sB")
            nc.vector.tensor_mul(s_sb[:], fac[:], rden[:])

            # phi' = y * s (s broadcast across partitions via K=8 matmul)
            phis = []
            for hh in range(8):
                sbc = sbp.tile([128, NST], F32, tag="sbc")
                nc.tensor.matmul(sbc[:], lhsT=ind2_sb[:, hh], rhs=s_sb[:],
                                 start=True, stop=True)
                for mc in range(2):
                    phi_c = php.tile([128, NST], BF16, tag="phB")
                    nc.vector.tensor_mul(phi_c[:], ys[2 * hh + mc][:], sbc[:])
                    phis.append(phi_c)

            # attnT chunks: attnT[(h,dc)] = sum_mc ktv[h,mc,dc]^T phi[(h,mc)] + vssT
            ats = []
            for c in range(16):
                hh, dc = c // 2, c % 2
                at_ps = ap_.tile([128, NST], F32, tag="atps")
                for mc in range(2):
                    nc.tensor.matmul(at_ps[:], lhsT=ktv_sb[:, hh, mc, dc],
                                     rhs=phis[2 * hh + mc][:],
                                     start=(mc == 0), stop=False)
                for kc in range(KC):
                    nc.tensor.matmul(at_ps[:], lhsT=wvm_sb[:, kc, c * 128:(c + 1) * 128],
                                     rhs=xs_sb[:, kc],
                                     start=False, stop=(kc == KC - 1))
                at_sb = atp.tile([128, NST], BF16, tag="atB")
                nc.scalar.copy(at_sb[:], at_ps[:])
                ats.append(at_sb)

            # final projection per 128-node subtile + Lorentz lift
            for sn in range(NST // 128):
                o_ps = op.tile([128, D], F32, tag="ops")
                for c in range(16):
                    nc.tensor.matmul(o_ps[:], lhsT=ats[c][:, sn * 128:(sn + 1) * 128],
                                     rhs=fw_sb[:, c], start=(c == 0), stop=False)
                nc.tensor.matmul(o_ps[:], lhsT=ones_r_sb[:], rhs=fb_sb[:],
                                 start=False, stop=True)
                sq = zp.tile([128, D], F32, tag="sqB")
                ssum = stp.tile([128, 1], F32, tag="ssum")
                nc.scalar.activation(sq[:], o_ps[:], AF.Square,
                                     accum_out=ssum[:])
                tcol = stp.tile([128, 1], F32, tag="tcol")
                nc.scalar.activation(tcol[:], ssum[:], AF.Sqrt, bias=1.0)
                o_sb = obp.tile([128, 257], F32, tag="osb")
                nc.vector.tensor_copy(o_sb[:, 1:257], o_ps[:])
                nc.vector.tensor_copy(o_sb[:, 0:1], tcol[:])
                nc.sync.dma_start(out[nofs + sn * 128: nofs + (sn + 1) * 128, :],
                                  o_sb[:])


def _prep_inputs(query_input, source_input, Wq_w, Wq_b, Wk_w, Wk_b, Wv_w, Wv_b,
                 norm_scale, v_map_w, v_map_b, final_w, final_b):
    def pad_x(x):
        xt = np.zeros((KC * 128, N), np.float32)
        xt[0:257] = x.T
        xt[257] = 1.0
        return xt.reshape(KC, 128, N)

    def pad_w(w_flat, b_flat):
        wt = np.zeros((KC * 128, HD), np.float32)
        wt[0:257] = w_flat.T
        wt[257] = b_flat
        return wt.reshape(KC, 128, HD)

    xq = pad_x(np.asarray(query_input))
    xs = pad_x(np.asarray(source_input))
    wq_h = pad_w(np.asarray(Wq_w).reshape(HD, 257), np.asarray(Wq_b).reshape(HD))
    wk_h = pad_w(np.asarray(Wk_w).reshape(HD, 257), np.asarray(Wk_b).reshape(HD))
    wv_h = pad_w(np.asarray(Wv_w).reshape(HD, 257), np.asarray(Wv_b).reshape(HD))

    vm = np.asarray(v_map_w)
    # wvm_flat[h] = vm @ Wv_w[h]  -> [H, 256, 257]
    wvm_flat = np.einsum('od,hdi->hoi', vm, np.asarray(Wv_w))
    bvm = (np.asarray(Wv_b) @ vm.T + np.asarray(v_map_b)[None, :]).reshape(HD)
    wvm_h = pad_w(wvm_flat.reshape(HD, 257), bvm)

    fw_h = np.ascontiguousarray(np.asarray(final_w).T).reshape(16, 128, D)
    fb_h = np.asarray(final_b).reshape(1, D).astype(np.float32)

    s = abs(float(np.asarray(norm_scale))) + EPS
    eps_eff = EPS * s * s
    cons = np.full((8, 1), eps_eff, np.float32)

    ind = np.zeros((128, 8, 8), ml_dtypes.bfloat16)
    for hh in range(8):
        ind[:, hh, hh] = 1.0
    indr = np.eye(8, dtype=np.float32).reshape(1, 8, 8)
    ind2 = np.zeros((8, 8, 128), np.float32)
    for hh in range(8):
        ind2[hh, hh, :] = 1.0

    common = {
        "wq": wq_h, "wk": wk_h, "wv": wv_h, "wvm": wvm_h,
        "fw": fw_h.astype(ml_dtypes.bfloat16), "fbias": fb_h,
        "ones_r": np.ones((1, 128), np.float32),
        "ones_c": np.ones((128, 8), np.float32),
        "ind": ind, "indr": indr, "ind2": ind2,
        "cons": cons,
    }
    in_maps = []
    for c in range(NCORES):
        m = dict(common)
        m["xqT"] = np.ascontiguousarray(xq[:, :, c * NCHUNK:(c + 1) * NCHUNK])
        m["xbT"] = np.ascontiguousarray(xs[:, :, c * NCHUNK:(c + 1) * NCHUNK])
        in_maps.append(m)
    return in_maps


def kernel(reps=1, **inputs):
    nc = _build(reps)
    in_maps = _prep_inputs(**inputs)
    res = run_bass_kernel_spmd(nc, in_maps, list(range(NCORES)))
    return np.concatenate([res.results[c]["out"] for c in range(NCORES)], axis=0)


# revision 3
# speedup vs baseline: 1.0658x; 1.0658x over previous
"""Trainium2 Bass kernel v3: node-sharded linear attention with AllReduce.

v2 (baseline) computed Phase A (k/v proj + ktv + sumk over ALL N=32768
nodes) redundantly on every core to avoid collectives. That made Phase A
~68.7G of the ~79.4G MACs per core.

v3 shards Phase A over nodes too: each core computes phi_k^T v and
sum(phi_k) partials over its OWN 4096 source nodes, then a single 2.07MB
AllReduce (ktv [H,2,2,128,128-lhsT-chunks] + sumk columns [128,16]) sums
partials across the 8 cores. Phase B (q proj, phi_q, numerator /
denominator, v_map path, final projection, Lorentz lift) is node-local
as before. Per-core MACs drop ~4.3x to ~18.3G.

Timing reps are python-unrolled (collectives cannot re-execute inside
rolled hardware loops on this stack); the graded kernel() entry point
builds the single-rep program.

All matmuls run as float32r (full PE rate at moving-dim>=256).
"""

import os
import numpy as np
import ml_dtypes
import concourse.bass as bass
import concourse.tile as tile
from concourse import bacc, mybir
from concourse.bass_utils import run_bass_kernel_spmd

F32 = mybir.dt.float32
F32R = mybir.dt.float32r
BF16 = mybir.dt.bfloat16
AF = mybir.ActivationFunctionType
ALU = mybir.AluOpType

NCORES = 8
N = 32768
NCHUNK = N // NCORES          # 4096 nodes per core
H = 8
D = 256
HD = H * D                    # 2048
KC = 3                        # contraction chunks: 384 = 3*128 (258 used)
EPS = 1e-6
CCG = 2048 + 64               # per-group AllGather width: 4-head ktv + sumk

_CACHE = {}


def _build(reps=1):
    key = (reps, os.environ.get("KT_DEBUG"), os.environ.get("KT_UNROLL"),
           os.environ.get("KT_NOSIDES"), os.environ.get("KT_WPA_BUFS"),
           os.environ.get("KT_SKIP_A"), os.environ.get("KT_SKIP_B"),
           os.environ.get("KT_SKIP_CC"), os.environ.get("KT_NTILES"),
           os.environ.get("KT_NST"))
    if key in _CACHE:
        return _CACHE[key]
    nc = bacc.Bacc("TRN2", target_bir_lowering=False, debug=False,
                   num_devices=NCORES)

    xqT = nc.dram_tensor("xqT", [KC, 128, NCHUNK], F32R, kind="ExternalInput").ap()
    xbT = nc.dram_tensor("xbT", [KC, 128, NCHUNK], F32R, kind="ExternalInput").ap()
    wq = nc.dram_tensor("wq", [KC, 128, HD], F32R, kind="ExternalInput").ap()
    wk = nc.dram_tensor("wk", [KC, 128, HD], F32R, kind="ExternalInput").ap()
    wv = nc.dram_tensor("wv", [KC, 128, HD], F32R, kind="ExternalInput").ap()
    wvm = nc.dram_tensor("wvm", [KC, 128, HD], F32R, kind="ExternalInput").ap()
    fw = nc.dram_tensor("fw", [16, 128, D], BF16, kind="ExternalInput").ap()
    fbias = nc.dram_tensor("fbias", [1, D], F32R, kind="ExternalInput").ap()
    ones_r = nc.dram_tensor("ones_r", [1, 128], F32R, kind="ExternalInput").ap()
    ones_c = nc.dram_tensor("ones_c", [128, 8], F32R, kind="ExternalInput").ap()
    ind = nc.dram_tensor("ind", [128, 8, 8], BF16, kind="ExternalInput").ap()
    indr = nc.dram_tensor("indr", [1, 8, 8], F32R, kind="ExternalInput").ap()
    ind2 = nc.dram_tensor("ind2", [8, 8, 128], F32R, kind="ExternalInput").ap()
    cons = nc.dram_tensor("cons", [8, 1], F32, kind="ExternalInput").ap()
    out = nc.dram_tensor("out", [NCHUNK, 257], F32, kind="ExternalOutput").ap()
    dbg = (nc.dram_tensor("dbg", [128, CCW], F32, kind="ExternalOutput").ap()
           if os.environ.get("KT_DEBUG") else None)

    with tile.TileContext(nc) as tc:
        _body(nc, tc, reps, xqT, xbT, wq, wk, wv, wvm, fw, fbias,
              ones_r, ones_c, ind, indr, ind2, cons, out, dbg)
    nc.compile()
    _CACHE[key] = nc
    return nc


def _body(nc, tc, reps, xqT, xbT, wq, wk, wv, wvm, fw, fbias,
          ones_r, ones_c, ind, indr, ind2, cons, out, dbg=None):
    import contextlib
    stack = contextlib.ExitStack()
    with stack:
        cpool = stack.enter_context(tc.tile_pool(name="const", bufs=1))
        dpool = stack.enter_context(tc.tile_pool(name="dramcc", bufs=1,
                                                 space="DRAM"))

        ones_r_sb = cpool.tile([1, 128], F32R)
        nc.sync.dma_start(ones_r_sb[:], ones_r[:])
        ones_c_sb = cpool.tile([128, 8], F32R)
        nc.sync.dma_start(ones_c_sb[:], ones_c[:])
        ind_sb = cpool.tile([128, 8, 8], BF16)
        nc.sync.dma_start(ind_sb[:], ind[:])
        indr_sb = cpool.tile([1, 8, 8], F32R)
        nc.sync.dma_start(indr_sb[:], indr[:])
        ind2_sb = cpool.tile([8, 8, 128], F32R)
        nc.sync.dma_start(ind2_sb[:], ind2[:])
        fb_sb = cpool.tile([1, D], F32R)
        nc.sync.dma_start(fb_sb[:], fbias[:])
        eps_sb = cpool.tile([8, 1], F32)
        nc.sync.dma_start(eps_sb[:], cons[:])

        # phase A -> AllGather -> phase B carriers
        # ktv as lhsT chunks [m_loc, h, mc, dc, d_loc]. Phase A emits f32
        # partials per head-group; each group's partials AllGather while the
        # next group computes; local f32 sum + bf16 rounding follow.
        # bufs=2 so rep r+1's phase A does not WAR-stall on rep r's phase B
        # still reading the previous carriers.
        kpool = stack.enter_context(tc.tile_pool(name="kcar", bufs=2))
        gpool = stack.enter_context(tc.tile_pool(name="agrp", bufs=2))
        agsp = stack.enter_context(tc.tile_pool(name="agstg", bufs=2))
        agap = stack.enter_context(tc.tile_pool(name="agacc", bufs=2))

        # phase B weights are rep-invariant: load once, keep resident
        wq_sb = cpool.tile([128, KC, HD], F32R)
        nc.sync.dma_start(wq_sb[:], wq.rearrange("c p n -> p c n"))
        wvm_sb = cpool.tile([128, KC, HD], F32R)
        nc.sync.dma_start(wvm_sb[:], wvm.rearrange("c p n -> p c n"))
        fw_sb = cpool.tile([128, 16, D], BF16)
        nc.sync.dma_start(fw_sb[:], fw.rearrange("c p n -> p c n"))

        def group_done(r, g, ktv_g, sumk_g, ktv_sb, sumk_w):
            """Called by _phase_a when head-group g's partials are ready.
            AllGather (pure copy), NOT AllReduce: the CCE reduction datapath
            rounds (~6.6e-4 max rel err measured); summing locally in f32
            keeps the 8-partial reduction exact. Collectives cannot
            re-execute inside rolled loops and a Shared output allows a
            single writer inst, so each (rep, group) gets its own bounce."""
            if os.environ.get("KT_SKIP_CC"):
                nc.scalar.copy(
                    ktv_sb[:, g * 4:(g + 1) * 4].rearrange(
                        "p h mc dc dl -> p (h mc dc dl)"), ktv_g[:])
                nc.scalar.copy(
                    sumk_w[:, g * 8:(g + 1) * 8].rearrange("p c h -> p (c h)"),
                    sumk_g[:])
                return
            cc_in = dpool.tile([128, CCG], F32, tag=f"ccin{r}_{g}",
                               name=f"ccin{r}_{g}")
            cc_out = dpool.tile([NCORES * 128, CCG], F32, addr_space="Shared",
                                tag=f"ccout{r}_{g}", name=f"ccout{r}_{g}")
            nc.sync.dma_start(cc_in[:, 0:2048], ktv_g[:])
            nc.sync.dma_start(cc_in[:, 2048:CCG], sumk_g[:])
            nc.gpsimd.collective_compute(
                "AllGather", ALU.bypass,
                replica_groups=[list(range(NCORES))],
                ins=[cc_in[:]], outs=[cc_out[:]])
            # local exact-f32 sum of the 8 gathered partials. One folded
            # DMA per chunk pulls all 8 ranks' [128, QW] slices (ranks
            # stack on the partition axis of cc_out), then 7 DVE adds.
            ktv_flat = ktv_sb[:, g * 4:(g + 1) * 4].rearrange(
                "p h mc dc dl -> p (h mc dc dl)")
            sumk_flat = sumk_w[:, g * 8:(g + 1) * 8].rearrange(
                "p c h -> p (c h)")
            cc_rk = cc_out.rearrange("(r p) w -> p r w", r=NCORES)
            QW = CCG // 16
            for q in range(16):
                lo, hi = q * QW, (q + 1) * QW
                stg = agsp.tile([128, NCORES, QW], F32, tag="agstg")
                nc.sync.dma_start(stg[:], cc_rk[:, :, lo:hi])
                acc = agap.tile([128, QW], F32, tag="agacc")
                nc.vector.tensor_add(acc[:], stg[:, 0], stg[:, 1])
                for rk in range(2, NCORES - 1):
                    nc.vector.tensor_add(acc[:], acc[:], stg[:, rk])
                # last add rounds straight into the bf16 carriers
                if hi <= 2048:
                    nc.vector.tensor_add(ktv_flat[:, lo:hi], acc[:],
                                         stg[:, NCORES - 1])
                else:
                    sp = 2048 - lo
                    nc.vector.tensor_add(ktv_flat[:, lo:2048], acc[:, 0:sp],
                                         stg[:, NCORES - 1, 0:sp])
                    nc.vector.tensor_add(sumk_flat[:], acc[:, sp:],
                                         stg[:, NCORES - 1, sp:])

        def rep_body(r):
            ktv_sb = kpool.tile([128, H, 2, 2, 128], BF16, tag="ktvsb",
                                name="ktv_sb")
            sumk_w = kpool.tile([128, 16, 8], BF16, tag="sumkw",
                                name="sumk_w")
            if not os.environ.get("KT_SKIP_A"):
                _phase_a(nc, tc, xbT, wk, wv, ones_c_sb, indr_sb, gpool,
                         lambda g, kg, sg: group_done(r, g, kg, sg,
                                                      ktv_sb, sumk_w))
            else:
                # timing-only ablation: garbage-init the carriers
                wsrc = wk.rearrange("c p n -> p c n")
                for g in range(2):
                    ktv_g = gpool.tile([128, 2048], F32, tag="ktvg")
                    sumk_g = gpool.tile([128, 64], F32, tag="sumkg")
                    nc.gpsimd.dma_start(ktv_g[:], wsrc[:, g])
                    nc.gpsimd.dma_start(sumk_g[:], wsrc[:, 2, 0:64])
                    group_done(r, g, ktv_g, sumk_g, ktv_sb, sumk_w)
            if not os.environ.get("KT_SKIP_B"):
                _phase_b(nc, tc, xqT, xbT, wq_sb, wvm_sb, fw_sb, fb_sb,
                         ones_r_sb, ind_sb, ind2_sb, eps_sb,
                         ktv_sb, sumk_w, out)

        # For_i is only legal when the collective is skipped or reps==1;
        # timing builds use python-unrolled reps (KT_UNROLL)
        if os.environ.get("KT_UNROLL") or reps == 1:
            for r in range(reps):
                rep_body(r)
        else:
            assert os.environ.get("KT_SKIP_CC"), (
                "rolled reps>1 require KT_SKIP_CC (collectives can't loop)")
            with tc.For_i(0, reps, name="reploop"):
                rep_body(0)


def _phase_a(nc, tc, xsT, wk, wv, ones_c_sb, indr_sb, gpool, group_done):
    """Per head-group (4 heads): project k/v for own nodes, accumulate ktv in
    PSUM and sumk in SBUF, reduce/transpose, emit f32 partials via
    group_done(g, ktv_g[128,2048], sumk_g[128,64]) so the group's AllGather
    overlaps the next group's compute."""
    import contextlib
    sd = None if os.environ.get("KT_NOSIDES") else "left"
    wpb = int(os.environ.get("KT_WPA_BUFS", 2))
    with contextlib.ExitStack() as st:
        apool = st.enter_context(tc.tile_pool(name="accA", bufs=2, side=sd))
        wpA = st.enter_context(tc.tile_pool(name="wA", bufs=wpb, side=sd))
        xp = st.enter_context(tc.tile_pool(name="xA", bufs=3, side=sd))
        zp = st.enter_context(tc.tile_pool(name="zA", bufs=2, side=sd))
        yp = st.enter_context(tc.tile_pool(name="yA", bufs=2, side=sd))
        scrp = st.enter_context(tc.tile_pool(name="scrA", bufs=2, side=sd))
        stp = st.enter_context(tc.tile_pool(name="stA", bufs=4, side=sd))
        php = st.enter_context(tc.tile_pool(name="phA", bufs=3, side=sd))
        vp = st.enter_context(tc.tile_pool(name="vA", bufs=3, side=sd))
        pk = st.enter_context(tc.tile_pool(name="psAk", bufs=1, space="PSUM"))
        pp = st.enter_context(tc.tile_pool(name="psAp", bufs=3, space="PSUM"))
        psk = st.enter_context(tc.tile_pool(name="psAs", bufs=1, space="PSUM"))

        def ktv_mms(ktv_ps, phi, v_sb, first, last):
            # ktv[h][m,d] += phi[:, h*256+mc*128]^T v[:, h*256:+256]
            # PSUM: start=True clears has_written for the WHOLE bank (= one
            # hh's 512 cols here), so only the mc=0 group may clear; mc=1's
            # first write lands on cleared bits and overwrites correctly.
            for hh in range(4):
                for mc in range(2):
                    nc.tensor.matmul(
                        ktv_ps[:, hh, mc * 256: mc * 256 + 256],
                        lhsT=phi[:, hh * 256 + mc * 128: hh * 256 + mc * 128 + 128],
                        rhs=v_sb[:, hh * 256: hh * 256 + 256],
                        start=(first and mc == 0), stop=last)

        ntiles = int(os.environ.get("KT_NTILES", NCHUNK // 128))
        wk_r = wk.rearrange("c p n -> p c n")
        wv_r = wv.rearrange("c p n -> p c n")
        for g in range(2):
            gofs = g * 1024
            wk_g = wpA.tile([128, KC, 1024], F32R, tag="wkg")
            nc.sync.dma_start(wk_g[:], wk_r[:, :, gofs:gofs + 1024])
            wv_g = wpA.tile([128, KC, 1024], F32R, tag="wvg")
            nc.sync.dma_start(wv_g[:], wv_r[:, :, gofs:gofs + 1024])
            sumk_acc = apool.tile([128, 1024], F32R, tag="ska")

            ktv_ps = pk.tile([128, 4, 512], F32)
            prev = None
            for t in range(ntiles):
                xs_sb = xp.tile([128, KC, 128], F32R, tag="xs")
                nc.sync.dma_start(
                    xs_sb[:],
                    xsT[:, :, t * 128:(t + 1) * 128].rearrange("c p n -> p c n"))

                ks_ps = []
                vs_ps = []
                for blk in range(2):
                    kp_t = pp.tile([128, 512], F32, tag="projA")
                    for c in range(KC):
                        nc.tensor.matmul(
                            kp_t[:], lhsT=xs_sb[:, c],
                            rhs=wk_g[:, c, blk * 512: blk * 512 + 512],
                            start=(c == 0), stop=(c == KC - 1))
                    ks_ps.append(kp_t)
                for blk in range(2):
                    vp_t = pp.tile([128, 512], F32, tag="projA")
                    for c in range(KC):
                        nc.tensor.matmul(
                            vp_t[:], lhsT=xs_sb[:, c],
                            rhs=wv_g[:, c, blk * 512: blk * 512 + 512],
                            start=(c == 0), stop=(c == KC - 1))
                    vs_ps.append(vp_t)

                # lagged ktv for the previous tile: keeps the PE busy while
                # this tile's z/y/phi chain runs on DVE/ACT
                if prev is not None:
                    ktv_mms(ktv_ps, prev[0], prev[1], prev[2] == 0, False)

                # z = relu(ks) + eps
                z = zp.tile([128, 1024], F32, tag="z")
                for blk in range(2):
                    nc.vector.tensor_scalar(
                        z[:, blk * 512:(blk + 1) * 512], ks_ps[blk][:],
                        0.0, EPS, ALU.max, ALU.add)
                # v copy to SBUF on DVE (ACT is the bottleneck engine)
                v_sb = vp.tile([128, 1024], BF16, tag="v")
                for blk in range(2):
                    nc.vector.tensor_copy(v_sb[:, blk * 512:(blk + 1) * 512],
                                          vs_ps[blk][:])

                # y = z^2 with per-head accumulated sums
                y = yp.tile([128, 1024], BF16, tag="y")
                sy = stp.tile([128, 4], F32, tag="sy")
                sy2 = stp.tile([128, 4], F32, tag="sy2")
                for hh in range(4):
                    sl = slice(hh * 256, hh * 256 + 256)
                    nc.scalar.activation(y[:, sl], z[:, sl], AF.Square,
                                         accum_out=sy[:, hh:hh + 1])
                for hh in range(4):
                    sl = slice(hh * 256, hh * 256 + 256)
                    scr = scrp.tile([128, 256], BF16, tag="y2scr")
                    nc.vector.tensor_mul(scr[:], y[:, sl], y[:, sl])
                    nc.vector.tensor_reduce(sy2[:, hh:hh + 1], scr[:],
                                            mybir.AxisListType.X, ALU.add)
                # factor = sqrt(sy / sy2)
                rec = stp.tile([128, 4], F32, tag="rec")
                nc.vector.reciprocal(rec[:], sy2[:])
                rat = stp.tile([128, 4], F32, tag="rat")
                nc.vector.tensor_mul(rat[:], sy[:], rec[:])
                fac = stp.tile([128, 4], F32, tag="fac")
                nc.scalar.activation(fac[:], rat[:], AF.Sqrt)

                phi = php.tile([128, 1024], BF16, tag="phi")
                for hh in range(4):
                    sl = slice(hh * 256, hh * 256 + 256)
                    nc.vector.tensor_scalar_mul(phi[:, sl], y[:, sl],
                                                fac[:, hh:hh + 1])
                # sumk accumulation
                if t == 0:
                    nc.scalar.copy(sumk_acc[:], phi[:])
                else:
                    nc.vector.tensor_add(sumk_acc[:], sumk_acc[:].bitcast(F32),
                                         phi[:])

                prev = (phi, v_sb, t)

            # tail: ktv for the final tile closes the accumulation group
            ktv_mms(ktv_ps, prev[0], prev[1], prev[2] == 0, True)

            # drain ktv psum into the f32 partial for this group's AllGather
            ktv_g = gpool.tile([128, 2048], F32, tag="ktvg")
            for hh in range(4):
                nc.scalar.copy(ktv_g[:, hh * 512:(hh + 1) * 512], ktv_ps[:, hh])

            # sumk partition-reduction for this group: [128, 1024] -> [1, 1024]
            srow = apool.tile([1, 1024], F32R, tag="srow")
            for blk in range(2):
                scr = psk.tile([128, 512], F32, tag="pscr")
                nc.tensor.matmul(
                    scr[0:8, :], lhsT=ones_c_sb[:],
                    rhs=sumk_acc[:, blk * 512:(blk + 1) * 512],
                    start=True, stop=True)
                nc.scalar.copy(srow[:, blk * 512:(blk + 1) * 512], scr[0:1, :])

            # transpose each 128-chunk of srow into sumk_g[:, j, :]
            # (col h(c) = sumk, rest zero), c = g*8 + j
            sumk_g = gpool.tile([128, 8, 8], F32, tag="sumkg")
            for j in range(8):
                hh = g * 4 + j // 2
                scr = psk.tile([128, 512], F32, tag="pscr")
                nc.tensor.matmul(scr[:, 0:8], lhsT=srow[:, j * 128:(j + 1) * 128],
                                 rhs=indr_sb[0:1, hh, :], start=True, stop=True)
                nc.scalar.copy(sumk_g[:, j], scr[:, 0:8])

            group_done(g, ktv_g,
                       sumk_g.rearrange("p c h -> p (c h)"))


def _phase_b(nc, tc, xqT, xbT, wq_sb, wvm_sb, fw_sb, fb_sb, ones_r_sb,
             ind_sb, ind2_sb, eps_sb, ktv_sb, sumk_w, out):
    import contextlib
    with contextlib.ExitStack() as st:
        sd = None if os.environ.get("KT_NOSIDES") else "right"
        xp = st.enter_context(tc.tile_pool(name="xB", bufs=2, side=sd))
        zp = st.enter_context(tc.tile_pool(name="zB", bufs=3, side=sd))
        yp = st.enter_context(tc.tile_pool(name="yB", bufs=17, side=sd))
        y2p = st.enter_context(tc.tile_pool(name="y2B", bufs=3, side=sd))
        stp = st.enter_context(tc.tile_pool(name="stB", bufs=2, side=sd))
        php = st.enter_context(tc.tile_pool(name="phB", bufs=17, side=sd))
        atp = st.enter_context(tc.tile_pool(name="atB", bufs=17, side=sd))
        obp = st.enter_context(tc.tile_pool(name="oB", bufs=3, side=sd))
        qp = st.enter_context(tc.tile_pool(name="psBq", bufs=2, space="PSUM"))
        sump = st.enter_context(tc.tile_pool(name="psBs", bufs=1, space="PSUM"))
        sbp = st.enter_context(tc.tile_pool(name="psBb", bufs=1, space="PSUM"))
        ap_ = st.enter_context(tc.tile_pool(name="psBa", bufs=2, space="PSUM"))
        op = st.enter_context(tc.tile_pool(name="psBo", bufs=1, space="PSUM"))

        NST = 256                      # supertile node count
        nst = int(os.environ.get("KT_NST", NCHUNK // NST))
        for stx in range(nst):
            nofs = stx * NST
            xq_sb = xp.tile([128, KC, NST], F32R, tag="xq")
            nc.sync.dma_start(
                xq_sb[:], xqT[:, :, nofs:nofs + NST].rearrange("c p n -> p c n"))
            xs_sb = xp.tile([128, KC, NST], F32R, tag="xsB")
            nc.sync.dma_start(
                xs_sb[:], xbT[:, :, nofs:nofs + NST].rearrange("c p n -> p c n"))

            sums_ps = sump.tile([8, 3, NST], F32, tag="sums")

            def sums_mms(c, y_c, y2):
                # streams 0 and 1 share a PSUM bank: only stream 0 clears
                # (start=True wipes the whole bank's has_written bits);
                # stream 1's first write overwrites-on-clear correctly.
                # stream 2 (den) needs the AllGathered sumk_w, so it runs in
                # its own pass after the supertile's AG-independent work —
                # the PE keeps busy while the collective lands.
                nc.tensor.matmul(sums_ps[:, 0], lhsT=ind_sb[:, c // 2], rhs=y_c[:],
                                 start=(c == 0), stop=(c == 15))
                nc.tensor.matmul(sums_ps[:, 1], lhsT=ind_sb[:, c // 2], rhs=y2[:],
                                 start=False, stop=(c == 15))

            ys = []
            pend = None
            for c in range(16):
                q_ps = qp.tile([128, NST], F32, tag="qps")
                for kc in range(KC):
                    nc.tensor.matmul(
                        q_ps[:], lhsT=wq_sb[:, kc, c * 128:(c + 1) * 128],
                        rhs=xq_sb[:, kc], start=(kc == 0), stop=(kc == KC - 1))
                # stats matmuls lag one chunk so the PE doesn't wait on the
                # z -> y -> y^2 chain of the chunk it just projected
                if pend is not None:
                    sums_mms(*pend)
                z = zp.tile([128, NST], F32, tag="zB")
                nc.vector.tensor_scalar(z[:], q_ps[:], 0.0, EPS, ALU.max, ALU.add)
                y_c = yp.tile([128, NST], BF16, tag="yB")
                nc.scalar.activation(y_c[:], z[:], AF.Square)
                y2 = y2p.tile([128, NST], BF16, tag="y2B")
                nc.vector.tensor_mul(y2[:], y_c[:], y_c[:])
                pend = (c, y_c, y2)
                ys.append(y_c)
            sums_mms(*pend)
            for c in range(16):
                nc.tensor.matmul(sums_ps[:, 2], lhsT=sumk_w[:, c], rhs=ys[c][:],
                                 start=(c == 0), stop=(c == 15))

            # stats on [8, NST]
            rec2 = stp.tile([8, NST], F32, tag="rec2")
            nc.vector.reciprocal(rec2[:], sums_ps[:, 1])
            rat = stp.tile([8, NST], F32, tag="ratB")
            nc.vector.tensor_mul(rat[:], sums_ps[:, 0], rec2[:])
            fac = stp.tile([8, NST], F32, tag="facB")
            nc.scalar.activation(fac[:], rat[:], AF.Sqrt)
            den = stp.tile([8, NST], F32, tag="den")
            nc.vector.tensor_mul(den[:], sums_ps[:, 2], fac[:])
            nc.vector.tensor_scalar_add(den[:], den[:], eps_sb[:])
            rden = stp.tile([8, NST], F32, tag="rden")
            nc.vector.reciprocal(rden[:], den[:])
            s_sb = stp.tile([8, NST], F32R, tag="sB")
            nc.vector.tensor_mul(s_sb[:], fac[:], rden[:])

            # phi' = y * s (s broadcast across partitions via K=8 matmul)
            phis = []
            for hh in range(8):
                sbc = sbp.tile([128, NST], F32, tag="sbc")
                nc.tensor.matmul(sbc[:], lhsT=ind2_sb[:, hh], rhs=s_sb[:],
                                 start=True, stop=True)
                for mc in range(2):
                    phi_c = php.tile([128, NST], BF16, tag="phB")
                    nc.vector.tensor_mul(phi_c[:], ys[2 * hh + mc][:], sbc[:])
                    phis.append(phi_c)

            # attnT chunks: attnT[(h,dc)] = sum_mc ktv[h,mc,dc]^T phi[(h,mc)] + vssT
            ats = []
            for c in range(16):
                hh, dc = c // 2, c % 2
                at_ps = ap_.tile([128, NST], F32, tag="atps")
                for mc in range(2):
                    nc.tensor.matmul(at_ps[:], lhsT=ktv_sb[:, hh, mc, dc],
                                     rhs=phis[2 * hh + mc][:],
                                     start=(mc == 0), stop=False)
                for kc in range(KC):
                    nc.tensor.matmul(at_ps[:], lhsT=wvm_sb[:, kc, c * 128:(c + 1) * 128],
                                     rhs=xs_sb[:, kc],
                                     start=False, stop=(kc == KC - 1))
                at_sb = atp.tile([128, NST], BF16, tag="atB")
                nc.scalar.copy(at_sb[:], at_ps[:])
                ats.append(at_sb)

            # final projection per 128-node subtile + Lorentz lift
            for sn in range(NST // 128):
                o_ps = op.tile([128, D], F32, tag="ops")
                for c in range(16):
                    nc.tensor.matmul(o_ps[:], lhsT=ats[c][:, sn * 128:(sn + 1) * 128],
                                     rhs=fw_sb[:, c], start=(c == 0), stop=False)
                nc.tensor.matmul(o_ps[:], lhsT=ones_r_sb[:], rhs=fb_sb[:],
                                 start=False, stop=True)
                sq = zp.tile([128, D], F32, tag="sqB")
                ssum = stp.tile([128, 1], F32, tag="ssum")
                nc.scalar.activation(sq[:], o_ps[:], AF.Square,
                                     accum_out=ssum[:])
                tcol = stp.tile([128, 1], F32, tag="tcol")
                nc.scalar.activation(tcol[:], ssum[:], AF.Sqrt, bias=1.0)
                o_sb = obp.tile([128, 257], F32, tag="osb")
                nc.vector.tensor_copy(o_sb[:, 1:257], o_ps[:])
                nc.vector.tensor_copy(o_sb[:, 0:1], tcol[:])
                nc.sync.dma_start(out[nofs + sn * 128: nofs + (sn + 1) * 128, :],
                                  o_sb[:])


def _prep_inputs(query_input, source_input, Wq_w, Wq_b, Wk_w, Wk_b, Wv_w, Wv_b,
                 norm_scale, v_map_w, v_map_b, final_w, final_b):
    def pad_x(x):
        xt = np.zeros((KC * 128, N), np.float32)
        xt[0:257] = x.T
        xt[257] = 1.0
        return xt.reshape(KC, 128, N)

    def pad_w(w_flat, b_flat):
        wt = np.zeros((KC * 128, HD), np.float32)
        wt[0:257] = w_flat.T
        wt[257] = b_flat
        return wt.reshape(KC, 128, HD)

    xq = pad_x(np.asarray(query_input))
    xs = pad_x(np.asarray(source_input))
    wq_h = pad_w(np.asarray(Wq_w).reshape(HD, 257), np.asarray(Wq_b).reshape(HD))
    wk_h = pad_w(np.asarray(Wk_w).reshape(HD, 257), np.asarray(Wk_b).reshape(HD))
    wv_h = pad_w(np.asarray(Wv_w).reshape(HD, 257), np.asarray(Wv_b).reshape(HD))

    vm = np.asarray(v_map_w)
    # wvm_flat[h] = vm @ Wv_w[h]  -> [H, 256, 257]
    wvm_flat = np.einsum('od,hdi->hoi', vm, np.asarray(Wv_w))
    bvm = (np.asarray(Wv_b) @ vm.T + np.asarray(v_map_b)[None, :]).reshape(HD)
    wvm_h = pad_w(wvm_flat.reshape(HD, 257), bvm)

    fw_h = np.ascontiguousarray(np.asarray(final_w).T).reshape(16, 128, D)
    fb_h = np.asarray(final_b).reshape(1, D).astype(np.float32)

    s = abs(float(np.asarray(norm_scale))) + EPS
    eps_eff = EPS * s * s
    cons = np.full((8, 1), eps_eff, np.float32)

    ind = np.zeros((128, 8, 8), ml_dtypes.bfloat16)
    for hh in range(8):
        ind[:, hh, hh] = 1.0
    indr = np.eye(8, dtype=np.float32).reshape(1, 8, 8)
    ind2 = np.zeros((8, 8, 128), np.float32)
    for hh in range(8):
        ind2[hh, hh, :] = 1.0

    common = {
        "wq": wq_h, "wk": wk_h, "wv": wv_h, "wvm": wvm_h,
        "fw": fw_h.astype(ml_dtypes.bfloat16), "fbias": fb_h,
        "ones_r": np.ones((1, 128), np.float32),
        "ones_c": np.ones((128, 8), np.float32),
        "ind": ind, "indr": indr, "ind2": ind2,
        "cons": cons,
    }
    in_maps = []
    for c in range(NCORES):
        m = dict(common)
        m["xqT"] = np.ascontiguousarray(xq[:, :, c * NCHUNK:(c + 1) * NCHUNK])
        m["xbT"] = np.ascontiguousarray(xs[:, :, c * NCHUNK:(c + 1) * NCHUNK])
        in_maps.append(m)
    return in_maps


def kernel(reps=1, **inputs):
    nc = _build(reps)
    in_maps = _prep_inputs(**inputs)
    res = run_bass_kernel_spmd(nc, in_maps, list(range(NCORES)))
    return np.concatenate([res.results[c]["out"] for c in range(NCORES)], axis=0)
